# revision 26
# baseline (speedup 1.0000x reference)
"""Causal self-attention (B=4, T=2048, D=1024, H=16) on 8 trn2 NeuronCores.

Sharding: tensor-parallel over heads — 2 heads per core. Each core computes
qkv projections for its 2 heads (from replicated x), causal attention, and a
partial output projection (its 128 rows of w_proj). Host sums the 8 partial
[S, D] outputs.

Per-core kernel formulation (everything bf16 into the PE, fp32 PSUM accum):
  xT [D, S] (host-pretransposed)  ->  qT, kT = w.T @ xT  [128, S]
  vT = wv.T @ xT, then PE-transposed into v blocks [S, 128]
  scores (transposed): sT[j, i] = kT-as-lhsT @ qT-as-rhs, per (b, head),
    2 heads packed in PE row-groups (K=64 each)
  p = exp(sT / 8) (ScalarE, causal-sliced), straddle diagonal masked by a
    lower-tri multiply
  oT[d, i] = [v_h | ones].T @ p accumulated over j tiles; the ones columns
    produce the softmax denominator rows for free
  normalize: reciprocal of gathered denominators, broadcast over 64
    partitions via a tiny K=2 matmul, multiply
  out partial = oT-as-lhsT @ w_proj-rows-as-rhs  [S, D] fp32 -> HBM
"""

import math

import numpy as np
import ml_dtypes

B, T, D, H = 4, 2048, 1024, 16
HD = D // H           # 64
S = B * T             # 8192
P = 128
KT = D // P           # 8 k-tiles over D
MC = S // 512         # 16 m-chunks of 512
NT = S // P           # 64 m-tiles of 128
JT = T // P           # 16 j-tiles per batch
NCH = T // 512        # 4 i-chunks per batch
N_CORES = 8

BFNP = ml_dtypes.bfloat16

_CACHE = {}


def _build_nc():
    import concourse.tile as tile
    import concourse.mybir as mybir
    from concourse import bacc

    BF = mybir.dt.bfloat16
    F32 = mybir.dt.float32
    Exp = mybir.ActivationFunctionType.Exp

    nc = bacc.Bacc("TRN2", num_devices=N_CORES)

    xT = nc.dram_tensor("xT", [D, S], BF, kind="ExternalInput").ap()
    wq = nc.dram_tensor("wq", [D, P], BF, kind="ExternalInput").ap()
    wk = nc.dram_tensor("wk", [D, P], BF, kind="ExternalInput").ap()
    wv = nc.dram_tensor("wv", [D, P], BF, kind="ExternalInput").ap()
    wp = nc.dram_tensor("wp", [P, D], BF, kind="ExternalInput").ap()
    maskt = nc.dram_tensor("maskt", [P, P], BF, kind="ExternalInput").ap()
    e01 = nc.dram_tensor("e01", [2, P], F32, kind="ExternalInput").ap()
    ident = nc.dram_tensor("ident", [P, P], BF, kind="ExternalInput").ap()
    out_p = nc.dram_tensor("out_p", [S, D], BF, kind="ExternalOutput").ap()

    with tile.TileContext(nc) as tc:
        with tc.tile_pool(name="singles", bufs=1) as singles:
            qT_sb = singles.tile([P, S], BF)
            kT_sb = singles.tile([P, S], BF)
            oT_sb = singles.tile([P, S], BF)
            # v blocks per m-tile: [v_h0 | ones | v_h1 | ones] (65-wide lhsTs)
            v_sb = singles.tile([P, NT, 130], BF)
            wq_sb = singles.tile([P, KT, P], BF)
            wk_sb = singles.tile([P, KT, P], BF)
            wv_sb = singles.tile([P, KT, P], BF)
            wp_sb = singles.tile([P, D], BF)
            mask_sb = singles.tile([P, P], BF)
            e01_sb = singles.tile([2, P], F32)
            id_sb = singles.tile([P, P], BF)
            vT_sb = singles.tile([P, S], BF)

            nc.sync.dma_start(out=wq_sb, in_=wq.rearrange("(kt p) n -> p kt n", p=P))
            nc.sync.dma_start(out=wk_sb, in_=wk.rearrange("(kt p) n -> p kt n", p=P))
            nc.sync.dma_start(out=wv_sb, in_=wv.rearrange("(kt p) n -> p kt n", p=P))
            nc.sync.dma_start(out=wp_sb, in_=wp)
            nc.sync.dma_start(out=mask_sb, in_=maskt)
            nc.sync.dma_start(out=e01_sb, in_=e01)
            nc.sync.dma_start(out=id_sb, in_=ident)
            nc.vector.memset(v_sb[:, :, 64:65], 1.0)
            nc.vector.memset(v_sb[:, :, 129:130], 1.0)

            xT_r = xT.rearrange("(kt p) m -> p kt m", p=P)

            # ---------------- Phase 1: QKV projections ----------------
            with (
                tc.tile_pool(name="xc_pool", bufs=3) as xpool,
                tc.tile_pool(name="ps1", bufs=2, space="PSUM") as ps1,
            ):
                for mc in range(MC):
                    sl = slice(mc * 512, (mc + 1) * 512)
                    xc = xpool.tile([P, KT, 512], BF, name="xc")
                    for kt in range(KT):
                        nc.sync.dma_start(out=xc[:, kt], in_=xT_r[:, kt, sl])

                    ps_q = ps1.tile([P, 512], F32, name="ps_q")
                    for kt in range(KT):
                        nc.tensor.matmul(ps_q, lhsT=wq_sb[:, kt], rhs=xc[:, kt],
                                         start=(kt == 0), stop=(kt == KT - 1))
                    nc.vector.tensor_copy(out=qT_sb[:, sl], in_=ps_q)

                    ps_k = ps1.tile([P, 512], F32, name="ps_k")
                    for kt in range(KT):
                        nc.tensor.matmul(ps_k, lhsT=wk_sb[:, kt], rhs=xc[:, kt],
                                         start=(kt == 0), stop=(kt == KT - 1))
                    nc.vector.tensor_copy(out=kT_sb[:, sl], in_=ps_k)

                    ps_v = ps1.tile([P, 512], F32, name="ps_v")
                    for kt in range(KT):
                        nc.tensor.matmul(ps_v, lhsT=wv_sb[:, kt], rhs=xc[:, kt],
                                         start=(kt == 0), stop=(kt == KT - 1))
                    nc.scalar.copy(out=vT_sb[:, sl], in_=ps_v)

                    for i in range(4):
                        mt = mc * 4 + i
                        ps_t = ps1.tile([P, P], BF, name="ps_t")
                        nc.tensor.transpose(
                            ps_t, vT_sb[:, mt * P:(mt + 1) * P], id_sb)
                        nc.vector.tensor_copy(out=v_sb[:, mt, 0:64],
                                              in_=ps_t[:, 0:64])
                        nc.vector.tensor_copy(out=v_sb[:, mt, 65:129],
                                              in_=ps_t[:, 64:128])

            # -------- Phase 2+3: attention with interleaved normalize+proj ------
            # PSUM budget (8 banks): s_0/s_1 bufs=2 (4) + av_0/av_1 bufs=1 (2)
            # + shared pjbc tag bufs=2 (2).
            with (
                tc.tile_pool(name="p_pool", bufs=6) as ppool,
                tc.tile_pool(name="dst_pool", bufs=4) as dstp,
                tc.tile_pool(name="g_pool", bufs=2) as gpool,
                tc.tile_pool(name="st_pool", bufs=8) as stp,
                tc.tile_pool(name="out_pool", bufs=4) as outp,
                tc.tile_pool(name="ps_s", bufs=3, space="PSUM") as ps2,
                tc.tile_pool(name="ps_av", bufs=1, space="PSUM") as avp,
            ):
                g_tiles = {}

                def attention_b(b, interleave=None):
                    g_b = gpool.tile([8, 512], F32, name="g_b")
                    g_tiles[b] = g_b
                    for c in range(NCH):
                        if interleave is not None:
                            interleave(c)
                        av_t = [avp.tile([P, 512], F32, name=f"av_{h}")
                                for h in (0, 1)]
                        pending_av = []  # software pipeline: AV one jt behind

                        def flush_av():
                            for args in pending_av:
                                nc.tensor.matmul(*args[0], **args[1])
                            pending_av.clear()

                        for jt in range(4 * c + 4):
                            diag = (jt // 4 == c)
                            off = jt * P - c * 512 if diag else 0
                            # both heads' scores in one 2-bank psum tile
                            s_ps = ps2.tile([P, 1024], F32, name="s_ps")
                            for h in (0, 1):
                                lk = kT_sb[h * 64:(h + 1) * 64,
                                           b * T + jt * P: b * T + (jt + 1) * P]
                                rq = qT_sb[h * 64:(h + 1) * 64,
                                           b * T + c * 512: b * T + (c + 1) * 512]
                                nc.tensor.matmul(s_ps[:, 512 * h: 512 * (h + 1)],
                                                 lhsT=lk, rhs=rq,
                                                 start=True, stop=True)
                            flush_av()
                            p_sb = ppool.tile([P, 1024], BF, name="p_sb")
                            scale = 1.0 / math.sqrt(HD)
                            if off < 172:
                                # single exp over both heads (junk span between
                                # the halves is never read downstream)
                                nc.scalar.activation(
                                    out=p_sb[:, off:1024], in_=s_ps[:, off:1024],
                                    func=Exp, scale=scale)
                            else:
                                for h in (0, 1):
                                    nc.scalar.activation(
                                        out=p_sb[:, 512 * h + off: 512 * (h + 1)],
                                        in_=s_ps[:, 512 * h + off: 512 * (h + 1)],
                                        func=Exp, scale=scale)
                            if diag:
                                for h in (0, 1):
                                    nc.vector.tensor_mul(
                                        out=p_sb[:, 512 * h + off: 512 * h + off + P],
                                        in0=p_sb[:, 512 * h + off: 512 * h + off + P],
                                        in1=mask_sb)
                            for h in (0, 1):
                                lv = v_sb[:, b * JT + jt, 65 * h: 65 * h + 65]
                                pending_av.append((
                                    (av_t[h][0:65, off:512],),
                                    dict(lhsT=lv,
                                         rhs=p_sb[:, 512 * h + off: 512 * (h + 1)],
                                         start=(jt == 0), stop=(jt == 4 * c + 3)),
                                ))
                        flush_av()
                        # oT (unnormalized) + denominator gather
                        for h in (0, 1):
                            avt = av_t[h]
                            nc.vector.tensor_copy(
                                out=oT_sb[h * 64:(h + 1) * 64,
                                          b * T + c * 512: b * T + (c + 1) * 512],
                                in_=avt[0:64])
                            # engine APs need 32-aligned partition base; stage
                            # at partition 0 then DMA-scatter into g_b
                            dstage = dstp.tile([1, 512], F32, name="dstage")
                            nc.vector.tensor_copy(out=dstage, in_=avt[64:65])
                            nc.sync.dma_start(out=g_b[c * 2 + h: c * 2 + h + 1, :],
                                              in_=dstage)
                    r_b = gpool.tile([8, 512], F32, name="r_b")
                    nc.vector.reciprocal(out=r_b, in_=g_b)
                    r2s = []
                    for c in range(NCH):
                        r2 = stp.tile([2, 512], F32, name="r2")
                        nc.sync.dma_start(out=r2, in_=r_b[c * 2: c * 2 + 2, :])
                        r2s.append(r2)
                    g_tiles[b] = r2s

                def norm_stage(b, c):
                    r2 = g_tiles[b][c]
                    bc_ps = avp.tile([P, 512], F32, name="bc_ps", tag="av_0")
                    nc.tensor.matmul(bc_ps, lhsT=e01_sb, rhs=r2,
                                     start=True, stop=True)
                    sl = slice(b * T + c * 512, b * T + (c + 1) * 512)
                    nc.vector.tensor_mul(out=oT_sb[:, sl], in0=oT_sb[:, sl],
                                         in1=bc_ps)

                def proj_stage(b, c):
                    for i in range(4):
                        mt = (b * T + c * 512) // P + i
                        ob = outp.tile([P, D], BF, name="ob")
                        for nch in range(2):
                            pj = avp.tile([P, 512], F32, name="pj",
                                          tag=f"av_{(2 * i + nch) % 2}")
                            nc.tensor.matmul(
                                pj, lhsT=oT_sb[:, mt * P:(mt + 1) * P],
                                rhs=wp_sb[:, nch * 512:(nch + 1) * 512],
                                start=True, stop=True)
                            if nch == 0:
                                nc.vector.tensor_copy(out=ob[:, 0:512], in_=pj)
                            else:
                                nc.scalar.copy(out=ob[:, 512:1024], in_=pj)
                        nc.sync.dma_start(out=out_p[mt * P:(mt + 1) * P, :],
                                          in_=ob)

                def make_interleave(bb):
                    def f(c):
                        norm_stage(bb, c)
                        if c >= 1:
                            proj_stage(bb, c - 1)
                    return f

                attention_b(0)
                for b in range(1, B):
                    attention_b(b, interleave=make_interleave(b - 1))
                    proj_stage(b - 1, NCH - 1)
                for c in range(NCH):
                    norm_stage(B - 1, c)
                    if c >= 1:
                        proj_stage(B - 1, c - 1)
                proj_stage(B - 1, NCH - 1)

    nc.compile()
    return nc


def _host_inputs(x, w_qkv, w_proj):
    x = np.asarray(x, dtype=np.float32)
    w_qkv = np.asarray(w_qkv, dtype=np.float32)
    w_proj = np.asarray(w_proj, dtype=np.float32)

    xT = np.ascontiguousarray(x.reshape(S, D).T).astype(BFNP)
    mask = np.triu(np.ones((P, P), np.float32)).astype(BFNP)  # [j, i]: 1 if j<=i
    e01 = np.zeros((2, P), np.float32)
    e01[0, :64] = 1.0
    e01[1, 64:] = 1.0
    ident = np.eye(P, dtype=np.float32).astype(BFNP)

    in_maps = []
    for core in range(N_CORES):
        cs = slice(core * P, (core + 1) * P)
        in_maps.append({
            "xT": xT,
            "wq": np.ascontiguousarray(w_qkv[:, core * P:(core + 1) * P]).astype(BFNP),
            "wk": np.ascontiguousarray(w_qkv[:, D + core * P: D + (core + 1) * P]).astype(BFNP),
            "wv": np.ascontiguousarray(w_qkv[:, 2 * D + core * P: 2 * D + (core + 1) * P]).astype(BFNP),
            "wp": np.ascontiguousarray(w_proj[cs, :]).astype(BFNP),
            "maskt": mask,
            "e01": e01,
            "ident": ident,
        })
    return in_maps


def run_spmd(x, w_qkv, w_proj, trace=False):
    """Compile (cached) + run on 8 cores. Returns (out [B,T,D] fp32, results)."""
    from concourse import bass_utils

    if "nc" not in _CACHE:
        _CACHE["nc"] = _build_nc()
    nc = _CACHE["nc"]

    in_maps = _host_inputs(x, w_qkv, w_proj)
    res = bass_utils.run_bass_kernel_spmd(
        nc, in_maps, core_ids=list(range(N_CORES)), trace=trace)

    acc = np.zeros((S, D), np.float32)
    for r in res.results:
        acc += np.asarray(r["out_p"]).astype(np.float32)
    return acc.reshape(B, T, D), res


def kernel(x, w_qkv, w_proj):
    out, _ = run_spmd(x, w_qkv, w_proj, trace=False)
    return out
